# revision 18
# baseline (speedup 1.0000x reference)
"""Morphological dilation (depthwise 3x3, additive SE) on 8 TRN2 NeuronCores.

out[b,c,h,w] = max_{dy,dx in {-1,0,1}} ( x[b,c,h+dy,w+dx] + k[c, (dy+1)*3+(dx+1)] )
with zero padding outside the image.

Sharding: batch -> 8 cores (1 image each). Per core, partitions = (h_half, c)
(2*64 = 128), free dim = flat (row-major, 226-wide padded rows).

The whole 9-term reduction runs as THREE custom-DVE passes per tile (T3A, a
hand-authored 1x-mode 8-block uop program):

  T3A: out[j] = max(in0[j-2]+s0, in0[j-1]+kL, in0[j]+s1, in1[1+j])

i.e. one pass folds a full window ROW (3 horizontal taps) into the running
max. The two off-alignment taps come from 1-beat swap-flop delay elements
(BYPASS+swap = emit previous beat's value, capture this beat's); the third
per-channel constant kL is latched from in1's first element by an init uop
(the C3-spill pattern). Streams are flat [R,226] runs, so row boundaries
need no AP tricks - the 2 pad columns absorb window wrap, and each output
buffer keeps [latch][2 garbage] slots ahead of its data.

Pass structure per chunk (output rows r0..r0+R):
  P1: o1 = row r0   taps (k0,k1,k2)        [T3N: no accumulator]
  P2: o2 = row r0+1 taps (k3,k4,k5) + o1   [T3A]
  P3: o3 = row r0+2 taps (k6,k7,k8) + o2   [T3A] -> DMA out

vs. the previous 8-pass 2x_1p add-max chain: 3 passes x 1 elem/cyc beats
8 passes x 2 elem/cyc, and the x2 (host-preshifted) input stream is gone -
HBM traffic drops from ~19.6MB to ~13.1MB per core. All DMas ride the sync
HWDGE queue: T3A reads both DVE ports every cycle, and one of them is
shared with GpSimd, so SWDGE (gpsimd) descriptor generation would contend.
"""

import numpy as np

_CACHE = {}

C = 64
H = 224
W = 224
WF = W + 2  # padded row width
HALF = 112
ROWS = HALF + 2  # per-half rows incl. 1-row halo each side
CHUNKS = (8, 48, 48, 8)
RMAX = max(CHUNKS)
BUF = 3 + RMAX * WF  # [latch][2 garbage][R*226 data]

_T3A_NAME = "T3A_WINMAX_ANT"
_T3N_NAME = "T3N_WINMAX_ANT"


def _register_winmax(name, with_acc):
    """Register a fused 3-tap window max-plus custom DVE op (idempotent).

    1x mode, 8 blocks, 2 uops:
      uop0 (init): consume in1[0], latch into blk4's swap flop (kL).
      uop1 (steady):
        T3A (with_acc): out[j] = max(x[j-2]+C0, x[j-1]+kL, x[j]+C1, acc[j])
                        with acc = in1[1:]
        T3N (no acc):   out[j] = max(x[j-2]+C0, x[j-1]+kL, x[j]+C1)
                        in1 = [P,1], just the latch element
    x = in0 (SRC_0). The 1- and 2-beat lookbacks are BYPASS+swap stages: the
    ALU emits CURR_SWAP_OUT (previous beat's capture) while the swap flop
    latches the complementary operand (this beat's value). out[0], out[1]
    are stale-flop garbage; callers land them in pad slots.
    """
    from concourse import dve_ops
    from concourse.dve_spec import Spec, Src0, Src1, C0, maxx
    from concourse.dve_uop import (
        AluInp,
        AluOp,
        DelayInp,
        DveOpSpec,
        ENABLE,
        InpSel,
        OutPath,
        OutSel,
        Trigger,
        UopConfig,
    )

    if name in dve_ops._SUB_OPCODE_FOR_NAME:
        return next(op for op in dve_ops.OPS if op.name == name)

    def _ref(in0, in1, s0, s1, imm2):
        x = np.asarray(in0, np.float32)
        P = x.shape[0]
        kl = np.asarray(in1[:, 0:1], np.float32)
        s0 = np.asarray(s0, np.float32).reshape(P, 1)
        s1 = np.asarray(s1, np.float32).reshape(P, 1)
        ninf = np.float32(-1e30)
        xm1 = np.concatenate([np.full((P, 1), ninf, np.float32), x[:, :-1]], 1)
        xm2 = np.concatenate([np.full((P, 2), ninf, np.float32), x[:, :-2]], 1)
        terms = [xm2 + s0, xm1 + kl, x + s1]
        if with_acc:
            terms.append(np.asarray(in1[:, 1:], np.float32))
        return np.maximum.reduce(terms)

    # Body is metadata only (rd1_en / no-C2 checks); the uops are the truth.
    spec = Spec(body=maxx(Src0 + C0, Src1), reference=_ref)

    def _build_uops():
        u0 = UopConfig()
        u0.enable_input(InpSel.SRC_1, 1)
        u0.require_inp1 = 1
        u0.trigger = (Trigger.COUNT, Trigger.NONE, Trigger.NONE)
        u0.repeat_count = 1
        u0.next_uop = (1, 0, 0)
        dp = u0.datapath_config
        for b in range(4):
            dp[b].pass_through_delay(0)  # chain0 <- lane1 = SRC_1 (kL)
        dp[4].enable_alu(AluOp.BYPASS, AluInp.PREV_DELAY_0, AluInp.PREV_DELAY_0)
        dp[4].swap_enable = ENABLE  # BYPASS(a): swap <- b = kL

        # lanes: 0=SRC_0(x), 1=SRC_1(acc), 2=CONST_0(s0), 3=CONST_1(s1)
        u1 = UopConfig()
        u1.enable_input(InpSel.SRC_0, 0)
        u1.enable_input(InpSel.CONST_0, 2)
        u1.enable_input(InpSel.CONST_1, 3)
        u1.require_inp0 = 1
        if with_acc:
            u1.enable_input(InpSel.SRC_1, 1)
            u1.require_inp1 = 1
        u1.trigger = (Trigger.SRC_TENSOR_DONE, Trigger.NONE, Trigger.NONE)
        dp = u1.datapath_config
        acc_chain = (0,) if with_acc else ()
        # blk0: t2 = x + C1; chains: [0<-acc,] 1<-C0, 2<-raw x
        dp[0].enable_alu(AluOp.ADD, AluInp.PREV_ALU_OUT, AluInp.PREV_DELAY_2)
        dp[0].pass_through_delay(*acc_chain, 1)
        dp[0].enable_delay_from_src(DelayInp.PREV_ALU_OUT, 2)
        # blk1: 1-beat delay: out = x(j-1), swap <- x(j)
        dp[1].enable_alu(AluOp.BYPASS, AluInp.CURR_SWAP_OUT, AluInp.PREV_DELAY_2)
        dp[1].swap_enable = ENABLE
        dp[1].pass_through_delay(*acc_chain, 1)
        dp[1].enable_delay_from_src(DelayInp.PREV_ALU_OUT, 3)  # chain3 <- t2
        # blk2: 1-beat delay: out = x(j-2), swap <- x(j-1)
        dp[2].enable_alu(AluOp.BYPASS, AluInp.CURR_SWAP_OUT, AluInp.PREV_ALU_OUT)
        dp[2].swap_enable = ENABLE
        dp[2].pass_through_delay(*acc_chain, 1, 3)
        dp[2].enable_delay_from_src(DelayInp.PREV_ALU_OUT, 4)  # chain4 <- x(j-1)
        # blk3: t0 = x(j-2) + C0
        dp[3].enable_alu(AluOp.ADD, AluInp.PREV_ALU_OUT, AluInp.PREV_DELAY_1)
        dp[3].pass_through_delay(*acc_chain, 3, 4)
        # blk4: tmid = x(j-1) + kL (kL persists in this blk's swap flop)
        dp[4].enable_alu(AluOp.ADD, AluInp.PREV_DELAY_4, AluInp.CURR_SWAP_OUT)
        dp[4].pass_through_delay(*acc_chain, 3)
        dp[4].enable_delay_from_src(DelayInp.PREV_ALU_OUT, 1)  # chain1 <- t0
        # blk5: m1 = max(tmid, t0)
        dp[5].enable_alu(AluOp.MAX, AluInp.PREV_ALU_OUT, AluInp.PREV_DELAY_1)
        dp[5].pass_through_delay(*acc_chain, 3)
        # blk6: m2 = max(m1, t2)
        dp[6].enable_alu(AluOp.MAX, AluInp.PREV_ALU_OUT, AluInp.PREV_DELAY_3)
        dp[6].pass_through_delay(*acc_chain)
        # blk7: out = max(m2, acc) / pass-through m2
        if with_acc:
            dp[7].enable_alu(AluOp.MAX, AluInp.PREV_ALU_OUT, AluInp.PREV_DELAY_0)
        else:
            dp[7].pass_through_alu()
        u1.enable_output(OutSel.ALU_OUT, OutPath.WR0_LO)
        return [u0, u1]

    class _WinMaxOp:
        subdim = False
        perf_en = {}
        uops_sha = {}

        def __init__(self):
            self.name = name
            self.spec = spec
            self._cache = {}

        def compile(self, ver):
            if ver in self._cache:
                return self._cache[ver]
            assert ver == "v3", "winmax ops authored for TRN2/v3"
            s = DveOpSpec(
                name=self.name,
                opcode=dve_ops.get_dve_sub_opcode(self.name),
                uops=_build_uops(),
                rd1_en=True,
                perf_max=0,
            )
            s.validate(ver)
            self._cache[ver] = s
            return s

    op = _WinMaxOp()
    dve_ops.OPS.append(op)
    dve_ops._SUB_OPCODE_FOR_NAME[op.name] = (
        dve_ops._CUSTOM_DVE_ROW_BASE + len(dve_ops.OPS) - 1
    )
    dve_ops.CUSTOM_DVE_SPECS[op.name] = spec
    assert dve_ops._SUB_OPCODE_FOR_NAME[op.name] < 0x20
    return op


def _build():
    import concourse.tile as tile
    import concourse.mybir as mybir
    from concourse import bacc

    f16 = mybir.dt.float16
    f32 = mybir.dt.float32

    t3a = _register_winmax(_T3A_NAME, with_acc=True)
    t3n = _register_winmax(_T3N_NAME, with_acc=False)

    nc = bacc.Bacc("TRN2", target_bir_lowering=False, debug=False)
    xe_t = nc.dram_tensor("xe", [128, ROWS * WF + 2], f16, kind="ExternalInput")
    k_t = nc.dram_tensor("k", [128, 9], f32, kind="ExternalInput")
    k16_t = nc.dram_tensor("k16", [128, 3], f16, kind="ExternalInput")
    o_t = nc.dram_tensor("out", [128, HALF * WF], f16, kind="ExternalOutput")

    starts = [sum(CHUNKS[:i]) for i in range(len(CHUNKS))]
    with tile.TileContext(nc) as tc:
        with (
            tc.tile_pool(name="const", bufs=1) as cpool,
            tc.tile_pool(name="xin", bufs=3) as xpool,
            tc.tile_pool(name="o", bufs=3) as opool,
        ):
            def load_chunk(ci):
                R, r0 = CHUNKS[ci], starts[ci]
                xe = xpool.tile([128, (RMAX + 2) * WF + 2], f16, tag="xe")
                n = (R + 2) * WF + 2
                nc.sync.dma_start(xe[:, 0:n], xe_t[:, r0 * WF : r0 * WF + n])
                return xe

            # Constants first on the sync queue (tiny), then the loads; a
            # scalar-queue const load proved to finish only after the big
            # sync-queue loads (cross-ring SDMA contention).
            kb = cpool.tile([128, 9], f32)
            k16 = cpool.tile([128, 3], f16)
            nc.sync.dma_start(kb[:], k_t[:])
            nc.sync.dma_start(k16[:], k16_t[:])
            loads = [load_chunk(0), load_chunk(1), load_chunk(2)]
            o1 = cpool.tile([128, BUF], f16)
            o2 = cpool.tile([128, BUF], f16)
            # Latch slots: o1[0]=k4, o2[0]=k7 (written once; the passes only
            # ever write cols 1.., so the slots persist); P1 latches k1
            # straight from the k16 tile. DVE copies, not tiny DMAs:
            # 2-byte-per-partition DMAs proved flaky (one partition on one
            # core read a stale latch).
            nc.vector.tensor_copy(o1[:, 0:1], k16[:, 1:2])
            nc.vector.tensor_copy(o2[:, 0:1], k16[:, 2:3])

            def t3(op, out, in0, in1, s0c, s1c):
                nc.vector._custom_dve(
                    op, out=out, in0=in0, in1=in1,
                    s0=kb[:, s0c : s0c + 1], s1=kb[:, s1c : s1c + 1],
                )

            for ci, R in enumerate(CHUNKS):
                r0 = starts[ci]
                if ci + 3 < len(CHUNKS):
                    loads.append(load_chunk(ci + 3))
                xe = loads[ci]
                o3 = opool.tile([128, BUF], f16, tag="o")
                N = 2 + R * WF
                t3(t3n, o1[:, 1 : 1 + N], xe[:, 0:N], k16[:, 0:1], 0, 2)
                t3(t3a, o2[:, 1 : 1 + N], xe[:, WF : WF + N], o1[:, 0 : N + 1], 3, 5)
                t3(t3a, o3[:, 1 : 1 + N], xe[:, 2 * WF : 2 * WF + N], o2[:, 0 : N + 1], 6, 8)
                nc.sync.dma_start(
                    o_t[:, r0 * WF : (r0 + R) * WF], o3[:, 3 : 3 + R * WF]
                )
    nc.finalize()
    return nc


LAST_RESULT = None


def kernel(x, kernel):
    """x: [8,64,224,224] f32; kernel: [1,64,9,1,1] f32 -> [8,64,224,224] f32."""
    global LAST_RESULT
    from concourse.bass_utils import run_bass_kernel_spmd

    if "nc" not in _CACHE:
        _CACHE["nc"] = _build()
    nc = _CACHE["nc"]

    B = x.shape[0]
    kf = np.ascontiguousarray(np.asarray(kernel, np.float32).reshape(C, 9))
    kb = np.concatenate([kf, kf], axis=0)  # [128, 9], partition p = half*64+c

    xp = np.zeros((B, C, H + 2, W + 2), np.float16)
    xp[:, :, 1 : H + 1, 1 : W + 1] = x
    # xe: [B, 128, 114*226+2] flat, partition p = half*64 + c
    xe3 = np.concatenate(
        [xp[:, :, 0:ROWS, :], xp[:, :, HALF : HALF + ROWS, :]], axis=1
    ).reshape(B, 128, ROWS * WF)
    xe = np.zeros((B, 128, ROWS * WF + 2), np.float16)
    xe[:, :, : ROWS * WF] = xe3
    # latch consts [k1, k4, k7] per partition, fp16
    k16 = np.ascontiguousarray(kb[:, [1, 4, 7]].astype(np.float16))

    in_maps = [{"xe": xe[b], "k": kb, "k16": k16} for b in range(B)]
    res = run_bass_kernel_spmd(nc, in_maps, core_ids=list(range(B)))
    LAST_RESULT = res
    out = np.stack([r["out"] for r in res.results], axis=0)  # [B, 128, 112*226]
    out = out.reshape(B, 2, C, HALF, WF)[:, :, :, :, 0:W]
    out = out.transpose(0, 2, 1, 3, 4).reshape(B, C, H, W)
    return out.astype(np.float32)


# revision 21
# speedup vs baseline: 1.0563x; 1.0563x over previous
"""Morphological dilation (depthwise 3x3, additive SE) on 8 TRN2 NeuronCores.

out[b,c,h,w] = max_{dy,dx in {-1,0,1}} ( x[b,c,h+dy,w+dx] + k[c, (dy+1)*3+(dx+1)] )
with zero padding outside the image.

Sharding: batch -> 8 cores (1 image each). Per core, partitions = (h_half, c)
(2*64 = 128), free dim = flat (row-major, 226-wide padded rows).

The whole 9-term reduction runs as THREE custom-DVE passes per tile (T3A, a
hand-authored 1x-mode 8-block uop program):

  T3A: out[j] = max(in0[j-2]+s0, in0[j-1]+kL, in0[j]+s1, in1[1+j])

i.e. one pass folds a full window ROW (3 horizontal taps) into the running
max. The two off-alignment taps come from 1-beat swap-flop delay elements
(BYPASS+swap = emit previous beat's value, capture this beat's); the third
per-channel constant kL is latched from in1's first element by an init uop
(the C3-spill pattern). Streams are flat [R,226] runs, so row boundaries
need no AP tricks - the 2 pad columns absorb window wrap, and each output
buffer keeps [latch][2 garbage] slots ahead of its data.

Pass structure per chunk (output rows r0..r0+R):
  P1: o1 = row r0   taps (k0,k1,k2)        [T3N: no accumulator]
  P2: o2 = row r0+1 taps (k3,k4,k5) + o1   [T3A]
  P3: o3 = row r0+2 taps (k6,k7,k8) + o2   [T3A] -> DMA out

vs. the previous 8-pass 2x_1p add-max chain: 3 passes x 1 elem/cyc beats
8 passes x 2 elem/cyc, and the x2 (host-preshifted) input stream is gone -
HBM traffic drops from ~19.6MB to ~13.1MB per core. All DMas ride the sync
HWDGE queue: T3A reads both DVE ports every cycle, and one of them is
shared with GpSimd, so SWDGE (gpsimd) descriptor generation would contend.
"""

import numpy as np

_CACHE = {}

C = 64
H = 224
W = 224
WF = W + 2  # padded row width
HALF = 112
ROWS = HALF + 2  # per-half rows incl. 1-row halo each side
CHUNKS = (8, 40, 44, 16, 4)
RMAX = max(CHUNKS)
BUF = 3 + RMAX * WF  # [latch][2 garbage][R*226 data]

_T3A_NAME = "T3A_WINMAX_ANT"
_T3N_NAME = "T3N_WINMAX_ANT"


def _register_winmax(name, with_acc):
    """Register a fused 3-tap window max-plus custom DVE op (idempotent).

    1x mode, 8 blocks, 2 uops:
      uop0 (init): consume in1[0], latch into blk4's swap flop (kL).
      uop1 (steady):
        T3A (with_acc): out[j] = max(x[j-2]+C0, x[j-1]+kL, x[j]+C1, acc[j])
                        with acc = in1[1:]
        T3N (no acc):   out[j] = max(x[j-2]+C0, x[j-1]+kL, x[j]+C1)
                        in1 = [P,1], just the latch element
    x = in0 (SRC_0). The 1- and 2-beat lookbacks are BYPASS+swap stages: the
    ALU emits CURR_SWAP_OUT (previous beat's capture) while the swap flop
    latches the complementary operand (this beat's value). out[0], out[1]
    are stale-flop garbage; callers land them in pad slots.
    """
    from concourse import dve_ops
    from concourse.dve_spec import Spec, Src0, Src1, C0, maxx
    from concourse.dve_uop import (
        AluInp,
        AluOp,
        DelayInp,
        DveOpSpec,
        ENABLE,
        InpSel,
        OutPath,
        OutSel,
        Trigger,
        UopConfig,
    )

    if name in dve_ops._SUB_OPCODE_FOR_NAME:
        return next(op for op in dve_ops.OPS if op.name == name)

    def _ref(in0, in1, s0, s1, imm2):
        x = np.asarray(in0, np.float32)
        P = x.shape[0]
        kl = np.asarray(in1[:, 0:1], np.float32)
        s0 = np.asarray(s0, np.float32).reshape(P, 1)
        s1 = np.asarray(s1, np.float32).reshape(P, 1)
        ninf = np.float32(-1e30)
        xm1 = np.concatenate([np.full((P, 1), ninf, np.float32), x[:, :-1]], 1)
        xm2 = np.concatenate([np.full((P, 2), ninf, np.float32), x[:, :-2]], 1)
        terms = [xm2 + s0, xm1 + kl, x + s1]
        if with_acc:
            terms.append(np.asarray(in1[:, 1:], np.float32))
        return np.maximum.reduce(terms)

    # Body is metadata only (rd1_en / no-C2 checks); the uops are the truth.
    spec = Spec(body=maxx(Src0 + C0, Src1), reference=_ref)

    def _build_uops():
        u0 = UopConfig()
        u0.enable_input(InpSel.SRC_1, 1)
        u0.require_inp1 = 1
        u0.trigger = (Trigger.COUNT, Trigger.NONE, Trigger.NONE)
        u0.repeat_count = 1
        u0.next_uop = (1, 0, 0)
        dp = u0.datapath_config
        for b in range(4):
            dp[b].pass_through_delay(0)  # chain0 <- lane1 = SRC_1 (kL)
        dp[4].enable_alu(AluOp.BYPASS, AluInp.PREV_DELAY_0, AluInp.PREV_DELAY_0)
        dp[4].swap_enable = ENABLE  # BYPASS(a): swap <- b = kL

        # lanes: 0=SRC_0(x), 1=SRC_1(acc), 2=CONST_0(s0), 3=CONST_1(s1)
        u1 = UopConfig()
        u1.enable_input(InpSel.SRC_0, 0)
        u1.enable_input(InpSel.CONST_0, 2)
        u1.enable_input(InpSel.CONST_1, 3)
        u1.require_inp0 = 1
        if with_acc:
            u1.enable_input(InpSel.SRC_1, 1)
            u1.require_inp1 = 1
        u1.trigger = (Trigger.SRC_TENSOR_DONE, Trigger.NONE, Trigger.NONE)
        dp = u1.datapath_config
        acc_chain = (0,) if with_acc else ()
        # blk0: t2 = x + C1; chains: [0<-acc,] 1<-C0, 2<-raw x
        dp[0].enable_alu(AluOp.ADD, AluInp.PREV_ALU_OUT, AluInp.PREV_DELAY_2)
        dp[0].pass_through_delay(*acc_chain, 1)
        dp[0].enable_delay_from_src(DelayInp.PREV_ALU_OUT, 2)
        # blk1: 1-beat delay: out = x(j-1), swap <- x(j)
        dp[1].enable_alu(AluOp.BYPASS, AluInp.CURR_SWAP_OUT, AluInp.PREV_DELAY_2)
        dp[1].swap_enable = ENABLE
        dp[1].pass_through_delay(*acc_chain, 1)
        dp[1].enable_delay_from_src(DelayInp.PREV_ALU_OUT, 3)  # chain3 <- t2
        # blk2: 1-beat delay: out = x(j-2), swap <- x(j-1)
        dp[2].enable_alu(AluOp.BYPASS, AluInp.CURR_SWAP_OUT, AluInp.PREV_ALU_OUT)
        dp[2].swap_enable = ENABLE
        dp[2].pass_through_delay(*acc_chain, 1, 3)
        dp[2].enable_delay_from_src(DelayInp.PREV_ALU_OUT, 4)  # chain4 <- x(j-1)
        # blk3: t0 = x(j-2) + C0
        dp[3].enable_alu(AluOp.ADD, AluInp.PREV_ALU_OUT, AluInp.PREV_DELAY_1)
        dp[3].pass_through_delay(*acc_chain, 3, 4)
        # blk4: tmid = x(j-1) + kL (kL persists in this blk's swap flop)
        dp[4].enable_alu(AluOp.ADD, AluInp.PREV_DELAY_4, AluInp.CURR_SWAP_OUT)
        dp[4].pass_through_delay(*acc_chain, 3)
        dp[4].enable_delay_from_src(DelayInp.PREV_ALU_OUT, 1)  # chain1 <- t0
        # blk5: m1 = max(tmid, t0)
        dp[5].enable_alu(AluOp.MAX, AluInp.PREV_ALU_OUT, AluInp.PREV_DELAY_1)
        dp[5].pass_through_delay(*acc_chain, 3)
        # blk6: m2 = max(m1, t2)
        dp[6].enable_alu(AluOp.MAX, AluInp.PREV_ALU_OUT, AluInp.PREV_DELAY_3)
        dp[6].pass_through_delay(*acc_chain)
        # blk7: out = max(m2, acc) / pass-through m2
        if with_acc:
            dp[7].enable_alu(AluOp.MAX, AluInp.PREV_ALU_OUT, AluInp.PREV_DELAY_0)
        else:
            dp[7].pass_through_alu()
        u1.enable_output(OutSel.ALU_OUT, OutPath.WR0_LO)
        return [u0, u1]

    class _WinMaxOp:
        subdim = False
        perf_en = {}
        uops_sha = {}

        def __init__(self):
            self.name = name
            self.spec = spec
            self._cache = {}

        def compile(self, ver):
            if ver in self._cache:
                return self._cache[ver]
            assert ver == "v3", "winmax ops authored for TRN2/v3"
            s = DveOpSpec(
                name=self.name,
                opcode=dve_ops.get_dve_sub_opcode(self.name),
                uops=_build_uops(),
                rd1_en=True,
                perf_max=0,
            )
            s.validate(ver)
            self._cache[ver] = s
            return s

    op = _WinMaxOp()
    dve_ops.OPS.append(op)
    dve_ops._SUB_OPCODE_FOR_NAME[op.name] = (
        dve_ops._CUSTOM_DVE_ROW_BASE + len(dve_ops.OPS) - 1
    )
    dve_ops.CUSTOM_DVE_SPECS[op.name] = spec
    assert dve_ops._SUB_OPCODE_FOR_NAME[op.name] < 0x20
    return op


def _build():
    import concourse.tile as tile
    import concourse.mybir as mybir
    from concourse import bacc

    f16 = mybir.dt.float16
    f32 = mybir.dt.float32

    t3a = _register_winmax(_T3A_NAME, with_acc=True)
    t3n = _register_winmax(_T3N_NAME, with_acc=False)

    nc = bacc.Bacc("TRN2", target_bir_lowering=False, debug=False)
    xe_t = nc.dram_tensor("xe", [128, ROWS * WF + 2], f16, kind="ExternalInput")
    k_t = nc.dram_tensor("k", [128, 9], f32, kind="ExternalInput")
    o_t = nc.dram_tensor("out", [128, HALF * WF], f16, kind="ExternalOutput")

    starts = [sum(CHUNKS[:i]) for i in range(len(CHUNKS))]
    with tile.TileContext(nc) as tc:
        with (
            tc.tile_pool(name="const", bufs=1) as cpool,
            tc.tile_pool(name="xin", bufs=3) as xpool,
            tc.tile_pool(name="o", bufs=3) as opool,
        ):
            def load_chunk(ci):
                R, r0 = CHUNKS[ci], starts[ci]
                xe = xpool.tile([128, (RMAX + 2) * WF + 2], f16, tag="xe")
                n = (R + 2) * WF + 2
                nc.sync.dma_start(xe[:, 0:n], xe_t[:, r0 * WF : r0 * WF + n])
                return xe

            # One tiny const DMA first on the sync queue, then the loads; a
            # second const DMA measurably ripples L0/L1 completions (+~1.3us
            # each), and a scalar-queue const load proved to finish only
            # after the big sync-queue loads (cross-ring SDMA contention).
            kb = cpool.tile([128, 9], f32)
            nc.sync.dma_start(kb[:], k_t[:])
            loads = [load_chunk(0), load_chunk(1), load_chunk(2)]
            kaux = cpool.tile([128, 1], f16)
            o1 = cpool.tile([128, BUF], f16)
            o2 = cpool.tile([128, BUF], f16)
            # Latch slots: kaux=k1 (P1's in1), o1[0]=k4, o2[0]=k7 (written
            # once; the passes only ever write cols 1.., so the slots
            # persist). DVE copies, not tiny DMAs: 2-byte-per-partition DMAs
            # proved flaky (one partition on one core read a stale latch).
            nc.vector.tensor_copy(kaux[:], kb[:, 1:2])
            nc.vector.tensor_copy(o1[:, 0:1], kb[:, 4:5])
            nc.vector.tensor_copy(o2[:, 0:1], kb[:, 7:8])

            def t3(op, out, in0, in1, s0c, s1c):
                nc.vector._custom_dve(
                    op, out=out, in0=in0, in1=in1,
                    s0=kb[:, s0c : s0c + 1], s1=kb[:, s1c : s1c + 1],
                )

            for ci, R in enumerate(CHUNKS):
                r0 = starts[ci]
                if ci + 3 < len(CHUNKS):
                    loads.append(load_chunk(ci + 3))
                xe = loads[ci]
                o3 = opool.tile([128, BUF], f16, tag="o")
                N = 2 + R * WF
                t3(t3n, o1[:, 1 : 1 + N], xe[:, 0:N], kaux[:], 0, 2)
                t3(t3a, o2[:, 1 : 1 + N], xe[:, WF : WF + N], o1[:, 0 : N + 1], 3, 5)
                t3(t3a, o3[:, 1 : 1 + N], xe[:, 2 * WF : 2 * WF + N], o2[:, 0 : N + 1], 6, 8)
                nc.sync.dma_start(
                    o_t[:, r0 * WF : (r0 + R) * WF], o3[:, 3 : 3 + R * WF]
                )
    nc.finalize()
    return nc


LAST_RESULT = None


def kernel(x, kernel):
    """x: [8,64,224,224] f32; kernel: [1,64,9,1,1] f32 -> [8,64,224,224] f32."""
    global LAST_RESULT
    from concourse.bass_utils import run_bass_kernel_spmd

    if "nc" not in _CACHE:
        _CACHE["nc"] = _build()
    nc = _CACHE["nc"]

    B = x.shape[0]
    kf = np.ascontiguousarray(np.asarray(kernel, np.float32).reshape(C, 9))
    kb = np.concatenate([kf, kf], axis=0)  # [128, 9], partition p = half*64+c

    xp = np.zeros((B, C, H + 2, W + 2), np.float16)
    xp[:, :, 1 : H + 1, 1 : W + 1] = x
    # xe: [B, 128, 114*226+2] flat, partition p = half*64 + c
    xe3 = np.concatenate(
        [xp[:, :, 0:ROWS, :], xp[:, :, HALF : HALF + ROWS, :]], axis=1
    ).reshape(B, 128, ROWS * WF)
    xe = np.zeros((B, 128, ROWS * WF + 2), np.float16)
    xe[:, :, : ROWS * WF] = xe3
    in_maps = [{"xe": xe[b], "k": kb} for b in range(B)]
    res = run_bass_kernel_spmd(nc, in_maps, core_ids=list(range(B)))
    LAST_RESULT = res
    out = np.stack([r["out"] for r in res.results], axis=0)  # [B, 128, 112*226]
    out = out.reshape(B, 2, C, HALF, WF)[:, :, :, :, 0:W]
    out = out.transpose(0, 2, 1, 3, 4).reshape(B, C, H, W)
    return out.astype(np.float32)
